# revision 1
# baseline (speedup 1.0000x reference)
"""LEGOTransformer (moe_routing early-exit) Trainium2 Bass kernel.

Reference semantics: tokens run through block0 (layers 0,1), compute
logits0 = hb0 @ head_w.T; tokens whose max softmax prob >= 1e-4 exit and
keep logits0. Remaining tokens run block1 (layers 2,3) from hb0 and take
logits1 (last block always writes active tokens).

Device strategy (8 NeuronCores):
  Launch A (token-sharded, 512 tok/core): embedding rows -> 2 transformer
    layers, feature-major activations ([D, tok] in SBUF) so every matmul
    uses weights-as-stored as the stationary operand and no transposes are
    needed. LN stats (sum, sum-sq) via ones-vector matmuls (fp32); per-token
    scale/shift broadcast across partitions via tiny outer-product matmuls.
    Main matmuls in float32r (full PE rate, ~1e-4..8e-4 rel err measured).
  Launch B (vocab-sharded, 6283 vocab cols/core): logits = hb0 @ head_wT
    for all 4096 tokens, written directly PSUM->DRAM, with fused per-token
    running max and sum(exp(l - 16)) stats (fixed shift => partials combine
    on host by plain max/sum).
  Host: exit mask from stats (identical decision to reference's
    max softmax >= 1e-4 with ~5x margin); tokens that do not exit (none for
    this input distribution, but handled honestly) get block1 + their logits
    row recomputed on host in fp32 numpy and patched in.
"""

import sys

sys.path.insert(0, "/opt/trn_rl_repo")

from contextlib import ExitStack

import numpy as np

from concourse import bacc, tile, mybir
from concourse.bass_utils import run_bass_kernel_spmd

F32 = mybir.dt.float32
F32R = mybir.dt.float32r
AF = mybir.ActivationFunctionType
OP = mybir.AluOpType

VOCAB = 50257
DIM = 1024
DFF = 4096
T = 4096
NCORES = 8
TPC = T // NCORES          # tokens per core in launch A
VS = 6284                  # vocab cols per core in launch B (6284*8 = 50272 >= 50257;
                           # ragged tile width 140 keeps f32r matmul free dim even)
VPAD = VS * NCORES
NVT = 13                   # 12 full 512-wide tiles + one 140-wide
LN_EPS = 1e-5
MHAT = 16.0                # fixed exp shift for Z stats
THRESH = 1e-4

_cache = {}

# test-harness knobs (harness never touches these; defaults are production)
TRACE = False
LAST_EXEC_NS = {}
LAST_PROFILE = {}


def _vtiles():
    out = []
    for v in range(NVT):
        lo = v * 512
        w = min(512, VS - lo)
        out.append((lo, w))
    return out


# --------------------------------------------------------------------------
# Launch A: two transformer layers, feature-major
# --------------------------------------------------------------------------

def _build_A():
    nc = bacc.Bacc(None, target_bir_lowering=False)
    hT = nc.declare_dram_parameter("hT", [DIM, TPC], F32, isOutput=False)
    wv = nc.declare_dram_parameter("wv", [2, DIM, DIM], F32R, isOutput=False)
    wo = nc.declare_dram_parameter("wo", [2, DIM, DIM], F32R, isOutput=False)
    w1 = nc.declare_dram_parameter("w1", [2, DIM, DFF], F32R, isOutput=False)
    w2 = nc.declare_dram_parameter("w2", [2, DFF, DIM], F32R, isOutput=False)
    ln1s = nc.declare_dram_parameter("ln1s", [2, DIM], F32R, isOutput=False)
    ln1b = nc.declare_dram_parameter("ln1b", [2, DIM], F32R, isOutput=False)
    ln2s = nc.declare_dram_parameter("ln2s", [2, DIM], F32R, isOutput=False)
    ln2b = nc.declare_dram_parameter("ln2b", [2, DIM], F32R, isOutput=False)
    b1d = nc.declare_dram_parameter("b1", [2, DFF], F32, isOutput=False)
    b2d = nc.declare_dram_parameter("b2", [2, DIM], F32, isOutput=False)
    hbT = nc.declare_dram_parameter("hbT", [DIM, TPC], F32, isOutput=True)

    with tile.TileContext(nc) as tc, ExitStack() as ctx:
        p_h = ctx.enter_context(tc.tile_pool(name="p_h", bufs=1))
        p_act = ctx.enter_context(tc.tile_pool(name="p_act", bufs=2))
        p_tmp = ctx.enter_context(tc.tile_pool(name="p_tmp", bufs=1))
        p_g = ctx.enter_context(tc.tile_pool(name="p_g", bufs=1))
        p_w = ctx.enter_context(tc.tile_pool(name="p_w", bufs=3))
        p_sq = ctx.enter_context(tc.tile_pool(name="p_sq", bufs=2))
        p_st = ctx.enter_context(tc.tile_pool(name="p_st", bufs=2))
        p_c = ctx.enter_context(tc.tile_pool(name="p_c", bufs=1))
        p_mm = ctx.enter_context(tc.tile_pool(name="p_mm", bufs=4, space="PSUM"))
        p_bc = ctx.enter_context(tc.tile_pool(name="p_bc", bufs=2, space="PSUM"))
        p_s12 = ctx.enter_context(tc.tile_pool(name="p_s12", bufs=1, space="PSUM"))

        ones128 = p_c.tile([128, 1], F32, tag="ones")
        nc.gpsimd.memset(ones128[:], 1.0)
        eps_t = p_c.tile([1, 1], F32, tag="eps")
        nc.gpsimd.memset(eps_t[:], LN_EPS)
        ones_row_f = p_c.tile([1, TPC], F32, tag="ones_row_f")
        nc.gpsimd.memset(ones_row_f[:], 1.0)
        ones_row = p_c.tile([1, TPC], F32R, tag="ones_row")
        nc.vector.tensor_copy(ones_row[:], ones_row_f[:])

        # per-layer LN scale/bias rows, each a [1, DIM] partition-0 row
        sb_ln = {}
        for li in range(2):
            for which, sd_, bd_ in (("ln1", ln1s, ln1b), ("ln2", ln2s, ln2b)):
                ts = p_c.tile([1, DIM], F32R, tag=f"s_{which}_{li}", name=f"s_{which}_{li}")
                nc.sync.dma_start(ts[:], sd_[li : li + 1, :])
                tb = p_c.tile([1, DIM], F32R, tag=f"b_{which}_{li}", name=f"b_{which}_{li}")
                nc.sync.dma_start(tb[:], bd_[li : li + 1, :])
                sb_ln[(which, li)] = (ts, tb)
        b1_sb = {}
        b2_sb = {}
        for li in range(2):
            t1 = p_c.tile([128, DFF // 128], F32, tag=f"b1_{li}")
            nc.sync.dma_start(t1[:], b1d[li].rearrange("(m p) -> p m", p=128))
            b1_sb[li] = t1
            t2 = p_c.tile([128, DIM // 128], F32, tag=f"b2_{li}")
            nc.sync.dma_start(t2[:], b2d[li].rearrange("(m p) -> p m", p=128))
            b2_sb[li] = t2

        h_fm = p_h.tile([128, 8, TPC], F32, tag="h")
        for k in range(8):
            nc.sync.dma_start(h_fm[:, k, :], hT[k * 128 : (k + 1) * 128, :])

        def layernorm(src_fm, sb):
            s_t, b_t = sb
            """Returns new tile [128, 8, TPC] with LN(src) applied."""
            s1 = p_s12.tile([1, TPC], F32, tag="s1")
            s2 = p_s12.tile([1, TPC], F32, tag="s2")
            for k in range(8):
                nc.tensor.matmul(
                    s1[:], ones128[:], src_fm[:, k, :], start=(k == 0), stop=(k == 7)
                )
            for k in range(8):
                sq = p_sq.tile([128, TPC], F32, tag="sq")
                nc.vector.tensor_mul(sq[:], src_fm[:, k, :], src_fm[:, k, :])
                nc.tensor.matmul(
                    s2[:], ones128[:], sq[:], start=(k == 0), stop=(k == 7)
                )
            mu = p_st.tile([1, TPC], F32, tag="mu")
            nc.vector.tensor_scalar_mul(mu[:], s1[:], 1.0 / DIM)
            var = p_st.tile([1, TPC], F32, tag="var")
            nc.vector.tensor_scalar_mul(var[:], s2[:], 1.0 / DIM)
            musq = p_st.tile([1, TPC], F32, tag="musq")
            nc.vector.tensor_mul(musq[:], mu[:], mu[:])
            nc.vector.tensor_sub(var[:], var[:], musq[:])
            sd = p_st.tile([1, TPC], F32, tag="sd")
            nc.scalar.activation(sd[:], var[:], AF.Sqrt, bias=eps_t[:], scale=1.0)
            At = p_st.tile([1, TPC], F32R, tag="At")
            Bt = p_st.tile([1, TPC], F32R, tag="Bt")
            with nc.allow_low_precision(
                reason="rstd rows feed f32r matmuls; tf32 rounding is fine here"
            ):
                nc.vector.reciprocal(At[:], sd[:])
                # B = -mu * rstd
                nc.vector.scalar_tensor_tensor(
                    Bt[:], mu[:], -1.0, At[:], OP.mult, OP.mult
                )
            dst = p_act.tile([128, 8, TPC], F32R, tag="act")
            for m in range(8):
                msl = slice(m * 128, (m + 1) * 128)
                ab = p_bc.tile([128, TPC], F32, tag="bc")
                nc.tensor.matmul(
                    ab[:], s_t[:, msl], At[:], start=True, stop=True
                )
                bb = p_bc.tile([128, TPC], F32, tag="bc")
                # bb = s ox B + b ox ones  (two accumulating K=1 matmuls)
                nc.tensor.matmul(
                    bb[:], s_t[:, msl], Bt[:], start=True, stop=False
                )
                nc.tensor.matmul(
                    bb[:], b_t[:, msl], ones_row[:], start=False, stop=True
                )
                nc.vector.tensor_mul(dst[:, m, :], src_fm[:, m, :], ab[:])
                nc.vector.tensor_add(dst[:, m, :], dst[:, m, :], bb[:])
            return dst

        def matmul_stream(src_fm, wdram, kt, mt, epilogue):
            """dst[m] = sum_k w[k,m].T-style contraction, feature-major.

            src_fm: [128, kt, TPC] fp32; wdram: [kt*128, mt*128] f32r.
            epilogue(m, acc) consumes the accumulated PSUM tile.
            """
            for mg in range((mt + 3) // 4):
                mls = [ml for ml in range(4) if mg * 4 + ml < mt]
                accs = {}
                for k in range(kt):
                    wt = p_w.tile([128, 512], F32R, tag="wt")
                    nc.sync.dma_start(
                        wt[:, : len(mls) * 128],
                        wdram[
                            k * 128 : (k + 1) * 128,
                            mg * 512 : mg * 512 + len(mls) * 128,
                        ],
                    )
                    for ml in mls:
                        m = mg * 4 + ml
                        if k == 0:
                            accs[ml] = p_mm.tile(
                                [128, TPC], F32, tag="mm", name=f"acc{ml}"
                            )
                        nc.tensor.matmul(
                            accs[ml][:],
                            wt[:, ml * 128 : (ml + 1) * 128],
                            src_fm[:, k, :],
                            start=(k == 0),
                            stop=(k == kt - 1),
                        )
                for ml in mls:
                    epilogue(mg * 4 + ml, accs[ml])

        for li in range(2):
            # --- attention (seq len 1): h += LN1(h) @ wv @ wo ---
            a_fm = layernorm(h_fm, sb_ln[("ln1", li)])
            tmp_fm = p_tmp.tile([128, 8, TPC], F32R, tag="tmp")

            def ep_tmp(m, acc):
                nc.vector.tensor_copy(tmp_fm[:, m, :], acc[:])

            matmul_stream(a_fm, wv[li], 8, 8, ep_tmp)

            def ep_resid(m, acc):
                nc.vector.tensor_add(h_fm[:, m, :], h_fm[:, m, :], acc[:])

            matmul_stream(tmp_fm, wo[li], 8, 8, ep_resid)

            # --- mlp: h += gelu(LN2(h) @ w1 + b1) @ w2 + b2 ---
            m_fm = layernorm(h_fm, sb_ln[("ln2", li)])
            g_fm = p_g.tile([128, 32, TPC], F32R, tag="g")

            def ep_gelu(m, acc, li=li):
                nc.scalar.activation(
                    g_fm[:, m, :],
                    acc[:],
                    AF.Gelu_apprx_tanh,
                    bias=b1_sb[li][:, m : m + 1],
                    scale=1.0,
                )

            matmul_stream(m_fm, w1[li], 8, 32, ep_gelu)

            def ep_mlp(m, acc, li=li):
                nc.vector.scalar_tensor_tensor(
                    h_fm[:, m, :],
                    acc[:],
                    b2_sb[li][:, m : m + 1],
                    h_fm[:, m, :],
                    OP.add,
                    OP.add,
                )

            matmul_stream(g_fm, w2[li], 32, 8, ep_mlp)

        for k in range(8):
            nc.sync.dma_start(hbT[k * 128 : (k + 1) * 128, :], h_fm[:, k, :])

    nc.compile()
    return nc


# --------------------------------------------------------------------------
# Launch B: head matmul over all tokens, vocab shard, + softmax stats
# --------------------------------------------------------------------------

def _build_B():
    nc = bacc.Bacc(None, target_bir_lowering=False)
    hT = nc.declare_dram_parameter("hT", [DIM, T], F32R, isOutput=False)
    hw = nc.declare_dram_parameter("hw", [DIM, VS], F32R, isOutput=False)
    logits = nc.declare_dram_parameter("logits", [T, VS], F32, isOutput=True)
    zmax = nc.declare_dram_parameter("zmax", [128, T // 128], F32, isOutput=True)
    zsum = nc.declare_dram_parameter("zsum", [128, T // 128], F32, isOutput=True)
    NMT = T // 128  # 32 token tiles

    with tile.TileContext(nc) as tc, ExitStack() as ctx:
        p_h = ctx.enter_context(tc.tile_pool(name="p_h", bufs=1))
        p_hw = ctx.enter_context(tc.tile_pool(name="p_hw", bufs=2))
        p_esc = ctx.enter_context(tc.tile_pool(name="p_esc", bufs=2))
        p_ot = ctx.enter_context(tc.tile_pool(name="p_ot", bufs=4))
        p_strip = ctx.enter_context(tc.tile_pool(name="p_strip", bufs=1))
        p_c = ctx.enter_context(tc.tile_pool(name="p_c", bufs=1))
        p_mm = ctx.enter_context(tc.tile_pool(name="p_mm", bufs=6, space="PSUM"))

        neg16 = p_c.tile([128, 1], F32, tag="neg16")
        nc.gpsimd.memset(neg16[:], -MHAT)

        hT_sb = p_h.tile([128, 8, T], F32R, tag="hT")
        for k in range(8):
            nc.sync.dma_start(hT_sb[:, k, :], hT[k * 128 : (k + 1) * 128, :])

        maxstrip = p_strip.tile([128, NMT, NVT], F32, tag="maxs")
        sumstrip = p_strip.tile([128, NMT, NVT], F32, tag="sums")

        for v, (lo, wv_) in enumerate(_vtiles()):
            hwt = p_hw.tile([128, 8, 512], F32R, tag="hw")
            for k in range(8):
                nc.sync.dma_start(
                    hwt[:, k, :wv_], hw[k * 128 : (k + 1) * 128, lo : lo + wv_]
                )
            for mt in range(NMT):
                acc = p_mm.tile([128, 512], F32, tag="mm")
                for k in range(8):
                    nc.tensor.matmul(
                        acc[:, :wv_],
                        hT_sb[:, k, mt * 128 : (mt + 1) * 128],
                        hwt[:, k, :wv_],
                        start=(k == 0),
                        stop=(k == 7),
                    )
                ot = p_ot.tile([128, 512], F32, tag="ot")
                nc.vector.tensor_copy(ot[:, :wv_], acc[:, :wv_])
                nc.sync.dma_start(
                    logits[mt * 128 : (mt + 1) * 128, lo : lo + wv_], ot[:, :wv_]
                )
                nc.vector.reduce_max(
                    maxstrip[:, mt, v : v + 1], ot[:, :wv_], axis=mybir.AxisListType.X
                )
                esc = p_esc.tile([128, 512], F32, tag="esc")
                nc.scalar.activation(
                    esc[:, :wv_],
                    ot[:, :wv_],
                    AF.Exp,
                    bias=neg16[:],
                    scale=1.0,
                    accum_out=sumstrip[:, mt, v : v + 1],
                )

        zmax_sb = p_c.tile([128, NMT], F32, tag="zmax")
        zsum_sb = p_c.tile([128, NMT], F32, tag="zsum")
        for mt in range(NMT):
            nc.vector.reduce_max(
                zmax_sb[:, mt : mt + 1], maxstrip[:, mt, :], axis=mybir.AxisListType.X
            )
            nc.vector.reduce_sum(
                zsum_sb[:, mt : mt + 1], sumstrip[:, mt, :], axis=mybir.AxisListType.X
            )
        nc.sync.dma_start(zmax[:], zmax_sb[:])
        nc.sync.dma_start(zsum[:], zsum_sb[:])

    nc.compile()
    return nc


def _get(name):
    if name not in _cache:
        _cache[name] = _build_A() if name == "A" else _build_B()
    return _cache[name]


# --------------------------------------------------------------------------
# Host side
# --------------------------------------------------------------------------

def _gelu_tanh(x):
    return 0.5 * x * (1.0 + np.tanh(0.7978845608028654 * (x + 0.044715 * x * x * x)))


def _host_block1(hb, inputs):
    """Block-1 layers (li=2,3) + head, fp32 numpy, for non-exiting tokens."""
    hb = hb.astype(np.float32)
    for li in (2, 3):
        mu = hb.mean(-1, keepdims=True, dtype=np.float32)
        var = hb.var(-1, keepdims=True, dtype=np.float32)
        a = (hb - mu) / np.sqrt(var + LN_EPS)
        a = a * inputs["ln1_s"][li] + inputs["ln1_b"][li]
        hb = hb + (a @ inputs["wv"][li]) @ inputs["wo"][li]
        mu = hb.mean(-1, keepdims=True, dtype=np.float32)
        var = hb.var(-1, keepdims=True, dtype=np.float32)
        m = (hb - mu) / np.sqrt(var + LN_EPS)
        m = m * inputs["ln2_s"][li] + inputs["ln2_b"][li]
        hb = hb + _gelu_tanh(m @ inputs["w1"][li] + inputs["b1"][li]) @ inputs["w2"][
            li
        ] + inputs["b2"][li]
    return hb @ np.asarray(inputs["head_w"], np.float32).T


def kernel(**inputs):
    x = np.asarray(inputs["x"]).reshape(-1).astype(np.int64)
    emb = np.asarray(inputs["emb"], dtype=np.float32)
    head_w = np.asarray(inputs["head_w"], dtype=np.float32)
    f32c = lambda k: np.ascontiguousarray(np.asarray(inputs[k], dtype=np.float32))

    h0 = emb[x]  # [T, DIM]

    ncA = _get("A")
    wA = {
        "wv": f32c("wv")[:2],
        "wo": f32c("wo")[:2],
        "w1": f32c("w1")[:2],
        "w2": f32c("w2")[:2],
        "ln1s": f32c("ln1_s")[:2],
        "ln1b": f32c("ln1_b")[:2],
        "ln2s": f32c("ln2_s")[:2],
        "ln2b": f32c("ln2_b")[:2],
        "b1": f32c("b1")[:2],
        "b2": f32c("b2")[:2],
    }
    in_maps_A = []
    for c in range(NCORES):
        m = dict(wA)
        m["hT"] = np.ascontiguousarray(h0[c * TPC : (c + 1) * TPC].T)
        in_maps_A.append(m)
    resA = run_bass_kernel_spmd(
        ncA, in_maps_A, core_ids=list(range(NCORES)), trace=TRACE
    )
    if TRACE:
        LAST_EXEC_NS["A"] = resA.exec_time_ns
        LAST_PROFILE["A"] = resA
    hbT = np.concatenate(
        [resA.results[c]["hbT"] for c in range(NCORES)], axis=1
    )  # [DIM, T]

    hwT = np.zeros((DIM, VPAD), np.float32)
    hwT[:, :VOCAB] = head_w.T
    ncB = _get("B")
    in_maps_B = [
        {"hT": hbT, "hw": np.ascontiguousarray(hwT[:, c * VS : (c + 1) * VS])}
        for c in range(NCORES)
    ]
    resB = run_bass_kernel_spmd(
        ncB, in_maps_B, core_ids=list(range(NCORES)), trace=TRACE
    )
    if TRACE:
        LAST_EXEC_NS["B"] = resB.exec_time_ns
        LAST_PROFILE["B"] = resB

    out = np.empty((T, VOCAB), np.float32)
    zmax = np.empty((NCORES, T), np.float32)
    zsum = np.empty((NCORES, T), np.float32)
    for c in range(NCORES):
        L = resB.results[c]["logits"]
        w = min(VS, VOCAB - c * VS)
        out[:, c * VS : c * VS + w] = L[:, :w]
        zmax[c] = resB.results[c]["zmax"].T.reshape(T)
        zsum[c] = resB.results[c]["zsum"].T.reshape(T)

    M = zmax.max(0)
    Z = zsum.sum(0, dtype=np.float32)
    max_prob = np.exp(M - MHAT).astype(np.float32) / Z
    cont = ~(max_prob >= THRESH)
    if cont.any():
        idx = np.where(cont)[0]
        out[idx] = _host_block1(hbT.T[idx], inputs)

    return out.reshape(tuple(np.asarray(inputs["x"]).shape) + (VOCAB,))



# revision 2
# speedup vs baseline: 1.0081x; 1.0081x over previous
"""LEGOTransformer (moe_routing early-exit) Trainium2 Bass kernel — fused.

Single launch per core, token-sharded end-to-end (512 tokens/core):
  phase A: 2 transformer layers, feature-major activations. Host fuses
    wvo = diag(ln1_s) @ wv @ wo and w1s = diag(ln2_s) @ w1, so LN reduces to
    zhat = (h - mu) * rstd; the per-token mean is folded into the matmul
    chain as a rank-1 correction (-wbar (x) mu*rstd), so the only row
    broadcast needed is one ones (x) rstd outer-product matmul per LN.
    Weights stream in bf16 from DRAM pre-tiled so each m-group is one
    contiguous ~1MB DMA (8KB per partition) -> PE never starves.
  phase B: head matmul over the FULL vocab for the core's own 512 tokens
    (stationary = hb token tile, moving = head_w bf16 columns), logits
    written to DRAM in bf16 (host upcasts), with fused per-token running
    max and sum(exp(l - 16)) for the early-exit decision.
  Host: exit mask from stats (same decision as reference's
    max softmax >= 1e-4, ~4.8x margin on this input distribution); tokens
    that do not exit get block1 + their logits recomputed on host in fp32.
"""

import sys

sys.path.insert(0, "/opt/trn_rl_repo")

from contextlib import ExitStack

import numpy as np
import ml_dtypes

from concourse import bacc, tile, mybir
from concourse.bass_utils import run_bass_kernel_spmd

F32 = mybir.dt.float32
BF16 = mybir.dt.bfloat16
AF = mybir.ActivationFunctionType
OP = mybir.AluOpType
NPBF16 = ml_dtypes.bfloat16

VOCAB = 50257
DIM = 1024
DFF = 4096
T = 4096
NCORES = 8
TPC = T // NCORES          # 512 tokens per core
NK = DIM // 128            # 8 feature k-slices
NTT = TPC // 128           # 4 token tiles
NVT = 99                   # 98 x 512 + 1 x 256 vocab tiles
VPAD = 98 * 512 + 256      # 50432 >= 50257
LN_EPS = 1e-5
MHAT = 16.0
THRESH = 1e-4

_cache = {}

# test-harness knobs (harness never touches these; defaults are production)
TRACE = False
LAST_EXEC_NS = {}
LAST_PROFILE = {}


def _vt_width(vt):
    return 512 if vt < 98 else 256


# --------------------------------------------------------------------------
# Device kernel
# --------------------------------------------------------------------------

def _build():
    nc = bacc.Bacc(None, target_bir_lowering=False)
    hT = nc.declare_dram_parameter("hT", [NK, 128, TPC], F32, isOutput=False)
    hTb = nc.declare_dram_parameter("hTb", [NK, 128, TPC], BF16, isOutput=False)
    # weight chunks: [li, mg, 128(p=fin%128), k, 512(fout cols)]
    wvo_d = nc.declare_dram_parameter("wvo", [2, 2, 128, NK, 512], BF16, isOutput=False)
    w1s_d = nc.declare_dram_parameter("w1s", [2, 8, 128, NK, 512], BF16, isOutput=False)
    w2_d = nc.declare_dram_parameter("w2", [2, 2, 4, 128, 8, 512], BF16, isOutput=False)
    r1_d = nc.declare_dram_parameter("r1", [2, 128, NK], F32, isOutput=False)
    b1p_d = nc.declare_dram_parameter("b1p", [2, 128, 32], F32, isOutput=False)
    b2_d = nc.declare_dram_parameter("b2", [2, 128, NK], F32, isOutput=False)
    hw_d = nc.declare_dram_parameter("hw", [128, NVT, NK, 512], BF16, isOutput=False)
    logits_d = nc.declare_dram_parameter("logits", [TPC, VPAD], BF16, isOutput=True)
    zmax_d = nc.declare_dram_parameter("zmax", [128, NTT], F32, isOutput=True)
    zsum_d = nc.declare_dram_parameter("zsum", [128, NTT], F32, isOutput=True)
    hb_out = nc.declare_dram_parameter("hb_out", [NK, 128, TPC], F32, isOutput=True)

    with tile.TileContext(nc) as tc, ExitStack() as ctx:
        p_c = ctx.enter_context(tc.tile_pool(name="p_c", bufs=1))
        p_h = ctx.enter_context(tc.tile_pool(name="p_h", bufs=1))
        p_z = ctx.enter_context(tc.tile_pool(name="p_z", bufs=2))
        p_sq = ctx.enter_context(tc.tile_pool(name="p_sq", bufs=2))
        p_g = ctx.enter_context(tc.tile_pool(name="p_g", bufs=1))
        p_w = ctx.enter_context(tc.tile_pool(name="p_w", bufs=6))
        p_st = ctx.enter_context(tc.tile_pool(name="p_st", bufs=2))
        p_hw = ctx.enter_context(tc.tile_pool(name="p_hw", bufs=4))
        p_ot = ctx.enter_context(tc.tile_pool(name="p_ot", bufs=6))
        p_strip = ctx.enter_context(tc.tile_pool(name="p_strip", bufs=1))
        p_mm = ctx.enter_context(tc.tile_pool(name="p_mm", bufs=4, space="PSUM"))
        p_s12 = ctx.enter_context(tc.tile_pool(name="p_s12", bufs=1, space="PSUM"))
        p_bc = ctx.enter_context(tc.tile_pool(name="p_bc", bufs=1, space="PSUM"))

        # ---- h0 bf16 load first: LN1 stats are the kernel's entry point ----
        h_fm = p_h.tile([128, NK, TPC], F32, tag="h_fm")
        h_bf = p_h.tile([128, NK, TPC], BF16, tag="h_bf")
        for k in range(NK):
            nc.sync.dma_start(h_bf[:, k, :], hTb[k])

        # ---- constants ----
        c1024f = p_c.tile([128, 1], F32, tag="c1024f")
        nc.gpsimd.memset(c1024f[:], 1.0 / DIM)
        c1024 = p_c.tile([128, 1], BF16, tag="c1024")
        nc.vector.tensor_copy(c1024[:], c1024f[:])
        onesrowf = p_c.tile([1, 128], F32, tag="onesrowf")
        nc.gpsimd.memset(onesrowf[:], 1.0)
        onesrow = p_c.tile([1, 128], BF16, tag="onesrow")
        nc.vector.tensor_copy(onesrow[:], onesrowf[:])
        eps_t = p_c.tile([1, 1], F32, tag="eps")
        nc.gpsimd.memset(eps_t[:], LN_EPS)
        neg16 = p_c.tile([128, 1], F32, tag="neg16")
        nc.gpsimd.memset(neg16[:], -MHAT)

        for k in range(NK):
            nc.sync.dma_start(h_fm[:, k, :], hT[k])

        r1_sb = p_c.tile([128, 2, NK], F32, tag="r1")
        nc.gpsimd.dma_start(r1_sb[:], r1_d.rearrange("l p m -> p l m"))
        b1p_sb = p_c.tile([128, 2, 32], F32, tag="b1p")
        nc.gpsimd.dma_start(b1p_sb[:], b1p_d.rearrange("l p m -> p l m"))
        b2_sb = p_c.tile([128, 2, NK], F32, tag="b2")
        nc.gpsimd.dma_start(b2_sb[:], b2_d.rearrange("l p m -> p l m"))

        def layernorm(li, which):
            """Compute rstd broadcast tile ab [128,TPC], Bt=-mu*rstd row, and
            zhat (bf16, pre-scaled by rstd) from h_bf. Returns (zhat, Bt)."""
            s1 = p_s12.tile([1, TPC], F32, tag="s1", name=f"s1_{li}_{which}")
            s2 = p_s12.tile([1, TPC], F32, tag="s2", name=f"s2_{li}_{which}")
            for k in range(NK):
                nc.tensor.matmul(
                    s1[:], c1024[:], h_bf[:, k, :], start=(k == 0), stop=(k == NK - 1)
                )
            # mu ops issued before the s2 chain so they overlap it on DVE
            mu_sb = p_st.tile([1, TPC], F32, tag="mu_sb")
            nc.vector.tensor_copy(mu_sb[:], s1[:])
            musq = p_st.tile([1, TPC], F32, tag="musq")
            nc.vector.tensor_mul(musq[:], mu_sb[:], mu_sb[:])
            for k in range(NK):
                sq = p_sq.tile([128, TPC], BF16, tag="sq")
                nc.vector.tensor_mul(sq[:], h_bf[:, k, :], h_bf[:, k, :])
                nc.tensor.matmul(
                    s2[:], c1024[:], sq[:], start=(k == 0), stop=(k == NK - 1)
                )
            # var = E[x^2] - mu^2
            var = p_st.tile([1, TPC], F32, tag="var")
            nc.vector.tensor_sub(var[:], s2[:], musq[:])
            sd = p_st.tile([1, TPC], F32, tag="sd")
            nc.scalar.activation(sd[:], var[:], AF.Sqrt, bias=eps_t[:], scale=1.0)
            At = p_st.tile([1, TPC], BF16, tag="At")
            Bt = p_st.tile([1, TPC], BF16, tag="Bt")
            with nc.allow_low_precision(
                reason="rstd rows feed bf16 matmuls; bf16 rounding fine here"
            ):
                nc.vector.reciprocal(At[:], sd[:])
                # Bt = -mu * rstd
                nc.vector.scalar_tensor_tensor(
                    Bt[:], mu_sb[:], -1.0, At[:], OP.mult, OP.mult
                )
            # zhat = h*rstd - mu*rstd, complete LN output (row broadcasts via
            # two K=1 outer-product matmuls; no per-chain mu correction)
            ab = p_bc.tile([128, TPC], F32, tag="ab")
            nc.tensor.matmul(ab[:], onesrow[:], At[:], start=True, stop=True)
            bb = p_bc.tile([128, TPC], F32, tag="bb")
            nc.tensor.matmul(bb[:], onesrow[:], Bt[:], start=True, stop=True)
            zhat = p_z.tile([128, NK, TPC], BF16, tag="zhat")
            for k in range(NK):
                zt = p_sq.tile([128, TPC], F32, tag="zt")
                nc.vector.tensor_mul(zt[:], h_bf[:, k, :], ab[:])
                nc.vector.tensor_add(zhat[:, k, :], zt[:], bb[:])
            return zhat

        def matmul_stream(src, wdram_li, nmg, kt, chunk_shape, epilogue,
                          tag="wchunk"):
            """out[m] = sum_k W[k,m].T @ src[k], m-grouped.

            wdram_li[mg] yields the DRAM chunk [128, kt, 512] for m-group mg.
            epilogue(m, acc) consumes the accumulated PSUM tile.
            """
            for mg in range(nmg):
                wt = p_w.tile(chunk_shape, BF16, tag=tag)
                nc.sync.dma_start(wt[:], wdram_li(mg))
                accs = []
                for ml in range(4):
                    acc = p_mm.tile([128, TPC], F32, tag="mm", name=f"acc{ml}")
                    accs.append(acc)
                    for k in range(kt):
                        nc.tensor.matmul(
                            acc[:],
                            wt[:, k, ml * 128 : (ml + 1) * 128],
                            src[:, k, :],
                            start=(k == 0),
                            stop=(k == kt - 1),
                        )
                for ml in range(4):
                    epilogue(mg * 4 + ml, accs[ml])

        for li in range(2):
            # --- attention (seq len 1): h += zhat1 @ wvo + r1 ---
            zhat = layernorm(li, "ln1")

            def ep_attn(m, acc, li=li):
                nc.vector.scalar_tensor_tensor(
                    h_fm[:, m, :], acc[:], r1_sb[:, li, m : m + 1],
                    h_fm[:, m, :], OP.add, OP.add,
                )
                nc.vector.tensor_copy(h_bf[:, m, :], h_fm[:, m, :])

            matmul_stream(
                zhat, lambda mg, li=li: wvo_d[li, mg], 2, NK, [128, NK, 512],
                ep_attn,
            )

            # --- mlp: h += gelu(zhat2 @ w1s + b1p) @ w2 + b2 ---
            zhat = layernorm(li, "ln2")
            g_bf = p_g.tile([128, 32, TPC], BF16, tag="g")

            def ep_gelu(m, acc, li=li):
                nc.scalar.activation(
                    g_bf[:, m, :], acc[:], AF.Gelu_apprx_tanh,
                    bias=b1p_sb[:, li, m : m + 1], scale=1.0,
                )

            matmul_stream(
                zhat, lambda mg, li=li: w1s_d[li, mg], 8, NK, [128, NK, 512],
                ep_gelu,
            )

            def ep_mlp(m, acc, li=li):
                nc.vector.scalar_tensor_tensor(
                    h_fm[:, m, :], acc[:], b2_sb[:, li, m : m + 1],
                    h_fm[:, m, :], OP.add, OP.add,
                )
                nc.vector.tensor_copy(h_bf[:, m, :], h_fm[:, m, :])

            # w2: contraction over DFF = 32 k-slices, streamed in 4 chunks of 8
            for mg in range(2):
                accs = []
                for ml in range(4):
                    accs.append(p_mm.tile([128, TPC], F32, tag="mm", name=f"acc{ml}"))
                for kc in range(4):
                    wt = p_w.tile([128, 8, 512], BF16, tag="wchunk")
                    nc.sync.dma_start(wt[:], w2_d[li, mg, kc])
                    for ml in range(4):
                        for k8 in range(8):
                            k = kc * 8 + k8
                            nc.tensor.matmul(
                                accs[ml][:],
                                wt[:, k8, ml * 128 : (ml + 1) * 128],
                                g_bf[:, k, :],
                                start=(k == 0),
                                stop=(k == 31),
                            )
                for ml in range(4):
                    ep_mlp(mg * 4 + ml, accs[ml])

        # ship hb (fp32) for the (rare) host block1 fallback
        for k in range(NK):
            nc.sync.dma_start(hb_out[k], h_fm[:, k, :])

        # --- head: logits[t, v] for own 512 tokens x full vocab ---
        maxstrip = p_strip.tile([128, NTT, NVT], F32, tag="maxs")
        sumstrip = p_strip.tile([128, NTT, NVT], F32, tag="sums")
        # partial reductions every 25 vtiles so the final reduce is tiny
        NGRP = 4
        grp_bounds = [(0, 25), (25, 50), (50, 75), (75, NVT)]
        max2 = p_strip.tile([128, NTT, NGRP], F32, tag="max2")
        sum2 = p_strip.tile([128, NTT, NGRP], F32, tag="sum2")

        for vt in range(NVT):
            wv_ = _vt_width(vt)
            hwt = p_hw.tile([128, NK, 512], BF16, tag="hw")
            nc.sync.dma_start(hwt[:, :, :wv_], hw_d[:, vt, :, :wv_])
            for tt in range(NTT):
                acc = p_mm.tile([128, 512], F32, tag="mm", name="hacc")
                for k in range(NK):
                    nc.tensor.matmul(
                        acc[:, :wv_],
                        h_bf[:, k, tt * 128 : (tt + 1) * 128],
                        hwt[:, k, :wv_],
                        start=(k == 0),
                        stop=(k == NK - 1),
                    )
                ot = p_ot.tile([128, 512], BF16, tag="ot")
                nc.vector.tensor_copy(ot[:, :wv_], acc[:, :wv_])
                nc.gpsimd.dma_start(
                    logits_d[tt * 128 : (tt + 1) * 128, vt * 512 : vt * 512 + wv_],
                    ot[:, :wv_],
                )
                nc.vector.reduce_max(
                    maxstrip[:, tt, vt : vt + 1], ot[:, :wv_],
                    axis=mybir.AxisListType.X,
                )
                esc = p_sq.tile([128, 512], BF16, tag="esc")
                nc.scalar.activation(
                    esc[:, :wv_], ot[:, :wv_], AF.Exp,
                    bias=neg16[:], scale=1.0,
                    accum_out=sumstrip[:, tt, vt : vt + 1],
                )
            for g, (lo, hi) in enumerate(grp_bounds):
                if vt == hi - 1:
                    for tt in range(NTT):
                        nc.vector.reduce_max(
                            max2[:, tt, g : g + 1], maxstrip[:, tt, lo:hi],
                            axis=mybir.AxisListType.X,
                        )
                        nc.vector.reduce_sum(
                            sum2[:, tt, g : g + 1], sumstrip[:, tt, lo:hi],
                            axis=mybir.AxisListType.X,
                        )

        zmax_sb = p_c.tile([128, NTT], F32, tag="zmax")
        zsum_sb = p_c.tile([128, NTT], F32, tag="zsum")
        for tt in range(NTT):
            nc.vector.reduce_max(
                zmax_sb[:, tt : tt + 1], max2[:, tt, :], axis=mybir.AxisListType.X
            )
            nc.vector.reduce_sum(
                zsum_sb[:, tt : tt + 1], sum2[:, tt, :], axis=mybir.AxisListType.X
            )
        nc.sync.dma_start(zmax_d[:], zmax_sb[:])
        nc.sync.dma_start(zsum_d[:], zsum_sb[:])

    nc.compile()
    return nc


def _get():
    if "nc" not in _cache:
        _cache["nc"] = _build()
    return _cache["nc"]


# --------------------------------------------------------------------------
# Host side
# --------------------------------------------------------------------------

def _gelu_tanh(x):
    return 0.5 * x * (1.0 + np.tanh(0.7978845608028654 * (x + 0.044715 * x * x * x)))


def _host_block1(hb, inputs):
    """Block-1 layers (li=2,3) + head, fp32 numpy, for non-exiting tokens."""
    hb = hb.astype(np.float32)
    for li in (2, 3):
        mu = hb.mean(-1, keepdims=True, dtype=np.float32)
        var = hb.var(-1, keepdims=True, dtype=np.float32)
        a = (hb - mu) / np.sqrt(var + LN_EPS)
        a = a * inputs["ln1_s"][li] + inputs["ln1_b"][li]
        hb = hb + (a @ inputs["wv"][li]) @ inputs["wo"][li]
        mu = hb.mean(-1, keepdims=True, dtype=np.float32)
        var = hb.var(-1, keepdims=True, dtype=np.float32)
        m = (hb - mu) / np.sqrt(var + LN_EPS)
        m = m * inputs["ln2_s"][li] + inputs["ln2_b"][li]
        hb = hb + _gelu_tanh(m @ inputs["w1"][li] + inputs["b1"][li]) @ inputs["w2"][
            li
        ] + inputs["b2"][li]
    return hb @ np.asarray(inputs["head_w"], np.float32).T


def _prep_weights(inputs):
    """Host-side fusion + tiling of the per-layer weights (bf16 chunks)."""
    f32 = lambda k: np.asarray(inputs[k], dtype=np.float32)
    wvo_t = np.empty((2, 2, 128, NK, 512), NPBF16)
    w1s_t = np.empty((2, 8, 128, NK, 512), NPBF16)
    w2_t = np.empty((2, 2, 4, 128, 8, 512), NPBF16)
    r1 = np.empty((2, 128, NK), np.float32)
    b1p = np.empty((2, 128, 32), np.float32)
    b2s = np.empty((2, 128, NK), np.float32)

    def tile_w(w, nmg, kt):
        # [K*128, M] -> [mg, 128, k, 512] with chunk [p, k, c] = w[k*128+p, mg*512+c]
        kdim, mdim = w.shape
        r = w.reshape(kt, 128, nmg, 512)
        return np.ascontiguousarray(r.transpose(2, 1, 0, 3))

    for li in range(2):
        s1 = f32("ln1_s")[li]; b1b = f32("ln1_b")[li]
        s2 = f32("ln2_s")[li]; b2b = f32("ln2_b")[li]
        wv, wo = f32("wv")[li], f32("wo")[li]
        w1, w2 = f32("w1")[li], f32("w2")[li]
        wvo = (s1[:, None] * wv) @ wo
        wvo_b = wvo.astype(NPBF16)
        w1s = s2[:, None] * w1
        w1s_b = w1s.astype(NPBF16)
        w2_b = w2.astype(NPBF16)
        wvo_t[li] = tile_w(wvo_b, 2, NK)
        w1s_t[li] = tile_w(w1s_b, 8, NK)
        # w2 chunk [mg, kc, p, k8, c] = w2[(kc*8+k8)*128+p, mg*512+c]
        w2_t[li] = np.ascontiguousarray(
            w2_b.reshape(4, 8, 128, 2, 512).transpose(3, 0, 2, 1, 4)
        )
        r1[li] = (b1b @ wv @ wo).reshape(NK, 128).T
        b1p[li] = (b2b @ w1 + f32("b1")[li]).reshape(32, 128).T
        b2s[li] = f32("b2")[li].reshape(NK, 128).T

    return dict(wvo=wvo_t, w1s=w1s_t, w2=w2_t, r1=r1, b1p=b1p, b2=b2s)


def _prep_head(head_w):
    # DRAM layout uses 99 full 512-wide slots; the kernel reads only the
    # first 256 cols of the last slot.
    hw = np.zeros((DIM, NVT * 512), np.float32)
    hw[:, :VOCAB] = head_w.T
    # [128, vt, k, 512] with [p, vt, k, c] = hwT[k*128+p, vt*512+c]
    r = hw.reshape(NK, 128, NVT, 512)
    return np.ascontiguousarray(r.transpose(1, 2, 0, 3)).astype(NPBF16)


def kernel(**inputs):
    x = np.asarray(inputs["x"]).reshape(-1).astype(np.int64)
    emb = np.asarray(inputs["emb"], dtype=np.float32)
    head_w = np.asarray(inputs["head_w"], dtype=np.float32)

    h0 = emb[x]  # [T, DIM]
    wmaps = _prep_weights(inputs)
    hw_t = _prep_head(head_w)

    nc = _get()
    in_maps = []
    for c in range(NCORES):
        hT = np.ascontiguousarray(
            h0[c * TPC : (c + 1) * TPC].T.reshape(NK, 128, TPC)
        )
        m = dict(wmaps)
        m["hT"] = hT
        m["hTb"] = hT.astype(NPBF16)
        m["hw"] = hw_t
        in_maps.append(m)

    res = run_bass_kernel_spmd(nc, in_maps, core_ids=list(range(NCORES)), trace=TRACE)
    if TRACE:
        LAST_EXEC_NS["F"] = res.exec_time_ns
        LAST_PROFILE["F"] = res

    out = np.empty((T, VOCAB), np.float32)
    M = np.empty(T, np.float32)
    Z = np.empty(T, np.float32)
    for c in range(NCORES):
        L = res.results[c]["logits"]
        out[c * TPC : (c + 1) * TPC] = L[:, :VOCAB].astype(np.float32)
        # token t (within core) = tt*128 + p -> zmax[p, tt]
        M[c * TPC : (c + 1) * TPC] = (
            np.asarray(res.results[c]["zmax"], np.float32).T.reshape(TPC)
        )
        Z[c * TPC : (c + 1) * TPC] = (
            np.asarray(res.results[c]["zsum"], np.float32).T.reshape(TPC)
        )

    max_prob = np.exp(M - MHAT) / Z
    cont = ~(max_prob >= THRESH)
    if cont.any():
        hb = np.empty((T, DIM), np.float32)
        for c in range(NCORES):
            hb[c * TPC : (c + 1) * TPC] = (
                np.asarray(res.results[c]["hb_out"], np.float32)
                .reshape(DIM, TPC).T
            )
        idx = np.where(cont)[0]
        out[idx] = _host_block1(hb[idx], inputs)

    return out.reshape(tuple(np.asarray(inputs["x"]).shape) + (VOCAB,))
